# revision 12
# baseline (speedup 1.0000x reference)
"""Trainium2 Bass kernel for nn_ChoquetIntegralConstrained.

Computes: sigmoid((x @ w_eff) / weight_sum - thr) where w_eff is built from
(wc, wint) via the constraint transform, x is [16384, 8256] f32.

Strategy: pure data parallel over batch across 8 NeuronCores. Each core gets
2048 rows, streamed as 69 chunk DMAs alternating the two HWDGE rings, each
consumed by one fused DVE STT (out = x*w, accum_out = row-sum) so the Vector
engine tracks the HBM stream within one chunk (~2.4 us).

Rows are laid out 124 per tile on SBUF partitions 2..125 (not 128 on 0..127):
profiling/runtime traffic reproducibly stretches SDMA engines 0 and 15 (the
ports serving partitions {0-3,32-35} and {92-95,124-127}) by ~30-40 us on 1-3
of the 8 cores per run. With 124-row tiles those two engines carry 6 lines
per chunk instead of 8, giving them ~25% slack so the interference no longer
gates chunk completion; the other 14 engines pace the stream.

The weight vector is uploaded as a bf16 hi/lo pair (hi + lo == fp32 w to
~2^-18) and broadcast to all 128 partitions by the otherwise-idle
TensorEngine at full bf16 rate (ones[2,128] stationary x w2[2,N] moving ->
PSUM f32), then ACT copies PSUM->SBUF through a 4-bank staging pool so the
Vector engine starts ~5 us into the kernel. The tiny constraint transform on
the 8256 weights is done on the host in fp32 (identical elementwise semantics
to the reference).
"""

import sys

import numpy as np

sys.path.insert(0, "/opt/trn_rl_repo")

N_CRIT = 128
N_PAIRS = N_CRIT * (N_CRIT - 1) // 2  # 8128
D = N_CRIT + N_PAIRS  # 8256
BATCH = 16384
N_CORES = 8
ROWS_PER_CORE = BATCH // N_CORES  # 2048
P = 128  # SBUF partitions
BASE = 2  # first partition used for x rows
ROWS_T = 124  # rows per tile (partitions BASE..BASE+ROWS_T)
N_TILES = -(-ROWS_PER_CORE // ROWS_T)  # 17 (last tile has 64 rows)
LAST_ROWS = ROWS_PER_CORE - (N_TILES - 1) * ROWS_T  # 64
MIN_W = np.float32(1e-07)

# Column split of the 8256 weight/x columns into 4 pieces.
COLS = [2048, 2048, 2048, 2112]
OFFS = [0, 2048, 4096, 6144]
# The final tile's last chunk is split in two to shorten the tail STT.
LAST_SPLIT = 1056

_CACHE = {}


def _build_program():
    import concourse.tile as tile
    from concourse import bacc, mybir

    nc = bacc.Bacc(
        "TRN2",
        debug=False,
        target_bir_lowering=False,
        num_devices=N_CORES,
    )
    f32 = mybir.dt.float32
    bf16 = mybir.dt.bfloat16
    x_d = nc.dram_tensor("x", [ROWS_PER_CORE, D], f32, kind="ExternalInput").ap()
    w_d = nc.dram_tensor("w2", [2, D], bf16, kind="ExternalInput").ap()
    c_d = nc.dram_tensor("consts", [P, 2], f32, kind="ExternalInput").ap()
    y_d = nc.dram_tensor("y", [ROWS_T, N_TILES], f32, kind="ExternalOutput").ap()

    N_CHUNKS = N_TILES * 4 + 1  # 69 (last chunk split in two)
    NBUF = 16  # chunk pool depth: ~39 us of DMA-ahead decoupling
    EARLY = 12  # x chunks dispatched before the PE/ACT weight block
    MMCH = 512  # one PSUM bank per matmul output

    with tile.TileContext(nc) as tc:
        with (
            tc.tile_pool(name="xcp", bufs=NBUF) as xcp,
            tc.tile_pool(name="wp", bufs=1) as wp,
            tc.tile_pool(name="pp", bufs=4, space="PSUM") as pp,
        ):
            # Persistent tiles
            w_pieces = [
                wp.tile([P, COLS[i]], f32, name=f"w_sb{i}") for i in range(4)
            ]
            accq_t = wp.tile([P, N_CHUNKS], f32)
            acc_t = wp.tile([P, N_TILES], f32)
            y_t = wp.tile([P, N_TILES], f32)
            c_t = wp.tile([P, 2], f32)
            # STT must write a full-size out; a stride-0 broadcast AP over a
            # [P, 1] dummy absorbs it without SBUF cost.
            dummy = wp.tile([P, 1], f32)

            # w2 (bf16 hi/lo pair) rides the sync HWDGE ring FIRST (SWDGE has
            # a ~5 us cold start); ones memset on the Vector engine (idle
            # until its first STT). consts only feed the final sigmoid, so
            # they can take the slow gpsimd ring.
            w2_t = wp.tile([2, D], bf16)
            nc.sync.dma_start(out=w2_t[:], in_=w_d[:])
            ones_t = wp.tile([2, P], bf16)
            nc.vector.memset(ones_t[:], 1.0)
            nc.gpsimd.dma_start(out=c_t[:], in_=c_d[:])

            # x chunk DMAs alternate between the two HWDGE rings (ACT and SP).
            dma_engines = (nc.scalar, nc.sync)
            chunk_tiles = [None] * N_CHUNKS

            def chunk_geom(g):
                """-> (row_lo, n_rows, col_lo, col_hi, piece, piece_lo)"""
                if g < N_CHUNKS - 2:
                    t, q = divmod(g, 4)
                    rows = ROWS_T if t < N_TILES - 1 else LAST_ROWS
                    return t * ROWS_T, rows, OFFS[q], OFFS[q] + COLS[q], q, 0
                r0 = (N_TILES - 1) * ROWS_T
                if g == N_CHUNKS - 2:
                    return r0, LAST_ROWS, OFFS[3], OFFS[3] + LAST_SPLIT, 3, 0
                return r0, LAST_ROWS, OFFS[3] + LAST_SPLIT, D, 3, LAST_SPLIT

            def issue_chunk(g):
                r0, rows, lo, hi, _, _ = chunk_geom(g)
                x_c = xcp.tile([P, hi - lo], f32)
                chunk_tiles[g] = x_c
                dma_engines[g % 2].dma_start(
                    out=x_c[BASE : BASE + rows, :], in_=x_d[r0 : r0 + rows, lo:hi]
                )

            def consume_chunk(g):
                # Compute engines need 32-aligned partition bases, so the STT
                # spans all 128 partitions; the 4 rows the DMA never writes
                # (0, 1, 126, 127) carry harmless stale data that the y store
                # slices away.
                _, _, lo, hi, q, plo = chunk_geom(g)
                n = hi - lo
                nc.vector.scalar_tensor_tensor(
                    out=dummy.broadcast_to((P, n)),
                    in0=chunk_tiles[g][:],
                    scalar=1.0,
                    in1=w_pieces[q][:, plo : plo + n],
                    op0=mybir.AluOpType.mult,
                    op1=mybir.AluOpType.mult,
                    accum_out=accq_t[:, g : g + 1],
                )

            # Dispatch the first EARLY chunk DMAs before the weight block so
            # both HWDGE rings start streaming x at t~0 (per-engine program
            # order would otherwise park the ACT ring behind the PSUM->SBUF
            # copies below).
            for g in range(EARLY):
                issue_chunk(g)

            # Weight broadcast: ones[2,128] bf16 stationary, w2[2,N] bf16
            # moving -> PSUM[128,N] f32 = w_hi + w_lo (fp32-accurate), staged
            # through a 4-deep PSUM pool so the PE runs ahead and the ACT
            # copies to SBUF go back-to-back.
            for i in range(4):
                for j in range(0, COLS[i], MMCH):
                    n = min(MMCH, COLS[i] - j)
                    mm = pp.tile([P, MMCH], f32)
                    nc.tensor.matmul(
                        mm[:, 0:n],
                        ones_t[:],
                        w2_t[:, OFFS[i] + j : OFFS[i] + j + n],
                        start=True,
                        stop=True,
                    )
                    nc.scalar.copy(w_pieces[i][:, j : j + n], mm[:, 0:n])

            # Steady state: issue chunk g+EARLY, consume chunk g.
            for g in range(N_CHUNKS):
                if g + EARLY < N_CHUNKS:
                    issue_chunk(g + EARLY)
                consume_chunk(g)

            # Combine the partial sums. Tiles 0..15 reduce as soon as their
            # STTs are done; tile 16 (5 partials) reduces alone so the tail
            # after the last chunk STT is minimal.
            nc.vector.tensor_reduce(
                out=acc_t[:, 0 : N_TILES - 1],
                in_=accq_t[:, 0 : 4 * (N_TILES - 1)].rearrange(
                    "p (t q) -> p t q", q=4
                ),
                axis=mybir.AxisListType.X,
                op=mybir.AluOpType.add,
            )
            nc.vector.tensor_reduce(
                out=acc_t[:, N_TILES - 1 : N_TILES],
                in_=accq_t[:, 4 * (N_TILES - 1) : N_CHUNKS].rearrange(
                    "p (t q) -> p t q", q=5
                ),
                axis=mybir.AxisListType.X,
                op=mybir.AluOpType.add,
            )

            nc.scalar.activation(
                out=y_t[:],
                in_=acc_t[:],
                func=mybir.ActivationFunctionType.Sigmoid,
                bias=c_t[:, 1:2],
                scale=c_t[:, 0:1],
            )
            nc.sync.dma_start(out=y_d[:], in_=y_t[BASE : BASE + ROWS_T, :])

    nc.compile()
    return nc


def _get_program():
    if "nc" not in _CACHE:
        _CACHE["nc"] = _build_program()
    return _CACHE["nc"]


def _host_weight_prep(wc, wint, thr):
    """Mirror reference._constrained_weights + weight_sum in fp32 numpy."""
    wc = np.asarray(wc, dtype=np.float32)
    wint = np.asarray(wint, dtype=np.float32)
    wc_eff = np.where(wc < 0, MIN_W, wc)
    ii, jj = np.triu_indices(N_CRIT, k=1)
    lower = np.maximum(-wc_eff[:, ii], -wc_eff[:, jj])
    wint_eff = np.maximum(wint, lower)
    w_eff = np.concatenate([wc_eff, wint_eff], axis=1)  # [1, D]
    wsum = np.float32(wc_eff.sum(dtype=np.float32)) + np.float32(
        wint_eff.sum(dtype=np.float32)
    )
    inv_wsum = np.float32(1.0) / wsum
    neg_thr = -np.float32(np.asarray(thr).reshape(-1)[0])
    return w_eff, inv_wsum, neg_thr


def _make_in_maps(x, wc, wint, thr):
    import ml_dtypes

    x = np.ascontiguousarray(np.asarray(x, dtype=np.float32))
    w_eff, inv_wsum, neg_thr = _host_weight_prep(wc, wint, thr)
    # bf16 hi/lo split: hi + lo == w_eff to ~2^-18 relative accuracy; the PE
    # broadcast sums them in fp32 PSUM.
    w_hi = w_eff.astype(ml_dtypes.bfloat16)
    w_lo = (w_eff - w_hi.astype(np.float32)).astype(ml_dtypes.bfloat16)
    w2 = np.ascontiguousarray(np.concatenate([w_hi, w_lo], axis=0))  # [2, D]
    consts = np.empty((P, 2), dtype=np.float32)
    consts[:, 0] = inv_wsum
    consts[:, 1] = neg_thr
    return [
        {
            "x": np.ascontiguousarray(x[c * ROWS_PER_CORE : (c + 1) * ROWS_PER_CORE]),
            "w2": w2,
            "consts": consts,
        }
        for c in range(N_CORES)
    ]


def _gather(results):
    # y core tile is [ROWS_T, N_TILES]: y[p, t] = shard row t*ROWS_T + p
    # (the last tile only fills LAST_ROWS rows; the rest is padding).
    parts = [
        np.asarray(results[c]["y"]).T.reshape(-1)[:ROWS_PER_CORE]
        for c in range(N_CORES)
    ]
    return np.concatenate(parts).reshape(BATCH, 1).astype(np.float32)


def _run(x, wc, wint, thr, trace=False):
    from concourse import bass_utils

    nc = _get_program()
    in_maps = _make_in_maps(x, wc, wint, thr)
    res = bass_utils.run_bass_kernel_spmd(
        nc, in_maps, core_ids=list(range(N_CORES)), trace=trace
    )
    return _gather(res.results), res


def kernel(x, wc, wint, thr):
    out, _ = _run(x, wc, wint, thr, trace=False)
    return out


# revision 13
# speedup vs baseline: 1.6193x; 1.6193x over previous
"""Trainium2 Bass kernel for nn_ChoquetIntegralConstrained.

Computes: sigmoid((x @ w_eff) / weight_sum - thr) where w_eff is built from
(wc, wint) via the constraint transform, x is [16384, 8256] f32.

Strategy: pure data parallel over batch across 8 NeuronCores. Each core gets
2048 rows, streamed as 69 chunk DMAs alternating the two HWDGE rings, each
consumed by one fused DVE STT (out = x*w, accum_out = row-sum) so the Vector
engine tracks the HBM stream within one chunk (~2.4 us).

Rows are laid out 124 per tile on SBUF partitions 2..125 (not 128 on 0..127):
profiling/runtime traffic reproducibly stretches SDMA engines 0 and 15 (the
ports serving partitions {0-3,32-35} and {92-95,124-127}) by ~30-40 us on 1-3
of the 8 cores per run. With 124-row tiles those two engines carry 6 lines
per chunk instead of 8, giving them ~25% slack so the interference no longer
gates chunk completion; the other 14 engines pace the stream.

The weight vector is uploaded as a bf16 hi/lo pair (hi + lo == fp32 w to
~2^-18) and broadcast to all 128 partitions by the otherwise-idle
TensorEngine at full bf16 rate (ones[2,128] stationary x w2[2,N] moving ->
PSUM f32), then ACT copies PSUM->SBUF through a 4-bank staging pool so the
Vector engine starts ~5 us into the kernel. The tiny constraint transform on
the 8256 weights is done on the host in fp32 (identical elementwise semantics
to the reference).
"""

import sys

import numpy as np

sys.path.insert(0, "/opt/trn_rl_repo")

N_CRIT = 128
N_PAIRS = N_CRIT * (N_CRIT - 1) // 2  # 8128
D = N_CRIT + N_PAIRS  # 8256
BATCH = 16384
N_CORES = 8
ROWS_PER_CORE = BATCH // N_CORES  # 2048
P = 128  # SBUF partitions
BASE = 2  # first partition used for x rows
ROWS_T = 124  # rows per tile (partitions BASE..BASE+ROWS_T)
N_TILES = -(-ROWS_PER_CORE // ROWS_T)  # 17 (last tile has 64 rows)
LAST_ROWS = ROWS_PER_CORE - (N_TILES - 1) * ROWS_T  # 64
MIN_W = np.float32(1e-07)

# Column split of the 8256 weight/x columns into 4 pieces.
COLS = [2048, 2048, 2048, 2112]
OFFS = [0, 2048, 4096, 6144]
# The final tile's last chunk is split in two to shorten the tail STT.
LAST_SPLIT = 1056

_CACHE = {}


def _build_program():
    import concourse.tile as tile
    from concourse import bacc, mybir

    nc = bacc.Bacc(
        "TRN2",
        debug=False,
        target_bir_lowering=False,
        num_devices=N_CORES,
    )
    f32 = mybir.dt.float32
    bf16 = mybir.dt.bfloat16
    x_d = nc.dram_tensor("x", [ROWS_PER_CORE, D], f32, kind="ExternalInput").ap()
    w_d = nc.dram_tensor("w2", [2, D], bf16, kind="ExternalInput").ap()
    c_d = nc.dram_tensor("consts", [P, 2], f32, kind="ExternalInput").ap()
    y_d = nc.dram_tensor("y", [ROWS_T, N_TILES], f32, kind="ExternalOutput").ap()

    N_CHUNKS = N_TILES * 4 + 1  # 69 (last chunk split in two)
    NBUF = 16  # chunk pool depth: ~39 us of DMA-ahead decoupling
    EARLY = 12  # x chunks dispatched before the PE/ACT weight block
    MMCH = 512  # one PSUM bank per matmul output

    with tile.TileContext(nc) as tc:
        with (
            tc.tile_pool(name="xcp", bufs=NBUF) as xcp,
            tc.tile_pool(name="wp", bufs=1) as wp,
            tc.tile_pool(name="pp", bufs=4, space="PSUM") as pp,
        ):
            # Persistent tiles
            w_pieces = [
                wp.tile([P, COLS[i]], f32, name=f"w_sb{i}") for i in range(4)
            ]
            accq_t = wp.tile([P, N_CHUNKS], f32)
            acc_t = wp.tile([P, N_TILES], f32)
            y_t = wp.tile([P, N_TILES], f32)
            c_t = wp.tile([P, 2], f32)
            # STT must write a full-size out; a stride-0 broadcast AP over a
            # [P, 1] dummy absorbs it without SBUF cost.
            dummy = wp.tile([P, 1], f32)

            # w2 (bf16 hi/lo pair) rides the sync HWDGE ring FIRST (SWDGE has
            # a ~5 us cold start); ones memset on the Vector engine (idle
            # until its first STT). consts only feed the final sigmoid, so
            # they can take the slow gpsimd ring.
            w2_t = wp.tile([2, D], bf16)
            nc.sync.dma_start(out=w2_t[:], in_=w_d[:])
            ones_t = wp.tile([2, P], bf16)
            nc.vector.memset(ones_t[:], 1.0)
            nc.gpsimd.dma_start(out=c_t[:], in_=c_d[:])

            # x chunk DMAs ride SWDGE (gpsimd): HWDGE's 16-engine descriptor
            # swizzle only engages for offset-0 full-partition tiles, but the
            # partition-masked layout needs [2:126] writes.
            dma_engines = (nc.gpsimd, nc.gpsimd)
            chunk_tiles = [None] * N_CHUNKS

            def chunk_geom(g):
                """-> (row_lo, n_rows, col_lo, col_hi, piece, piece_lo)"""
                if g < N_CHUNKS - 2:
                    t, q = divmod(g, 4)
                    rows = ROWS_T if t < N_TILES - 1 else LAST_ROWS
                    return t * ROWS_T, rows, OFFS[q], OFFS[q] + COLS[q], q, 0
                r0 = (N_TILES - 1) * ROWS_T
                if g == N_CHUNKS - 2:
                    return r0, LAST_ROWS, OFFS[3], OFFS[3] + LAST_SPLIT, 3, 0
                return r0, LAST_ROWS, OFFS[3] + LAST_SPLIT, D, 3, LAST_SPLIT

            def issue_chunk(g):
                r0, rows, lo, hi, _, _ = chunk_geom(g)
                x_c = xcp.tile([P, hi - lo], f32)
                chunk_tiles[g] = x_c
                dma_engines[g % 2].dma_start(
                    out=x_c[BASE : BASE + rows, :], in_=x_d[r0 : r0 + rows, lo:hi]
                )

            def consume_chunk(g):
                # Compute engines need 32-aligned partition bases, so the STT
                # spans all 128 partitions; the 4 rows the DMA never writes
                # (0, 1, 126, 127) carry harmless stale data that the y store
                # slices away.
                _, _, lo, hi, q, plo = chunk_geom(g)
                n = hi - lo
                nc.vector.scalar_tensor_tensor(
                    out=dummy.broadcast_to((P, n)),
                    in0=chunk_tiles[g][:],
                    scalar=1.0,
                    in1=w_pieces[q][:, plo : plo + n],
                    op0=mybir.AluOpType.mult,
                    op1=mybir.AluOpType.mult,
                    accum_out=accq_t[:, g : g + 1],
                )

            # Dispatch the first EARLY chunk DMAs before the weight block so
            # both HWDGE rings start streaming x at t~0 (per-engine program
            # order would otherwise park the ACT ring behind the PSUM->SBUF
            # copies below).
            for g in range(EARLY):
                issue_chunk(g)

            # Weight broadcast: ones[2,128] bf16 stationary, w2[2,N] bf16
            # moving -> PSUM[128,N] f32 = w_hi + w_lo (fp32-accurate), staged
            # through a 4-deep PSUM pool so the PE runs ahead and the ACT
            # copies to SBUF go back-to-back.
            for i in range(4):
                for j in range(0, COLS[i], MMCH):
                    n = min(MMCH, COLS[i] - j)
                    mm = pp.tile([P, MMCH], f32)
                    nc.tensor.matmul(
                        mm[:, 0:n],
                        ones_t[:],
                        w2_t[:, OFFS[i] + j : OFFS[i] + j + n],
                        start=True,
                        stop=True,
                    )
                    nc.scalar.copy(w_pieces[i][:, j : j + n], mm[:, 0:n])

            # Steady state: issue chunk g+EARLY, consume chunk g.
            for g in range(N_CHUNKS):
                if g + EARLY < N_CHUNKS:
                    issue_chunk(g + EARLY)
                consume_chunk(g)

            # Combine the partial sums. Tiles 0..15 reduce as soon as their
            # STTs are done; tile 16 (5 partials) reduces alone so the tail
            # after the last chunk STT is minimal.
            nc.vector.tensor_reduce(
                out=acc_t[:, 0 : N_TILES - 1],
                in_=accq_t[:, 0 : 4 * (N_TILES - 1)].rearrange(
                    "p (t q) -> p t q", q=4
                ),
                axis=mybir.AxisListType.X,
                op=mybir.AluOpType.add,
            )
            nc.vector.tensor_reduce(
                out=acc_t[:, N_TILES - 1 : N_TILES],
                in_=accq_t[:, 4 * (N_TILES - 1) : N_CHUNKS].rearrange(
                    "p (t q) -> p t q", q=5
                ),
                axis=mybir.AxisListType.X,
                op=mybir.AluOpType.add,
            )

            nc.scalar.activation(
                out=y_t[:],
                in_=acc_t[:],
                func=mybir.ActivationFunctionType.Sigmoid,
                bias=c_t[:, 1:2],
                scale=c_t[:, 0:1],
            )
            nc.sync.dma_start(out=y_d[:], in_=y_t[BASE : BASE + ROWS_T, :])

    nc.compile()
    return nc


def _get_program():
    if "nc" not in _CACHE:
        _CACHE["nc"] = _build_program()
    return _CACHE["nc"]


def _host_weight_prep(wc, wint, thr):
    """Mirror reference._constrained_weights + weight_sum in fp32 numpy."""
    wc = np.asarray(wc, dtype=np.float32)
    wint = np.asarray(wint, dtype=np.float32)
    wc_eff = np.where(wc < 0, MIN_W, wc)
    ii, jj = np.triu_indices(N_CRIT, k=1)
    lower = np.maximum(-wc_eff[:, ii], -wc_eff[:, jj])
    wint_eff = np.maximum(wint, lower)
    w_eff = np.concatenate([wc_eff, wint_eff], axis=1)  # [1, D]
    wsum = np.float32(wc_eff.sum(dtype=np.float32)) + np.float32(
        wint_eff.sum(dtype=np.float32)
    )
    inv_wsum = np.float32(1.0) / wsum
    neg_thr = -np.float32(np.asarray(thr).reshape(-1)[0])
    return w_eff, inv_wsum, neg_thr


def _make_in_maps(x, wc, wint, thr):
    import ml_dtypes

    x = np.ascontiguousarray(np.asarray(x, dtype=np.float32))
    w_eff, inv_wsum, neg_thr = _host_weight_prep(wc, wint, thr)
    # bf16 hi/lo split: hi + lo == w_eff to ~2^-18 relative accuracy; the PE
    # broadcast sums them in fp32 PSUM.
    w_hi = w_eff.astype(ml_dtypes.bfloat16)
    w_lo = (w_eff - w_hi.astype(np.float32)).astype(ml_dtypes.bfloat16)
    w2 = np.ascontiguousarray(np.concatenate([w_hi, w_lo], axis=0))  # [2, D]
    consts = np.empty((P, 2), dtype=np.float32)
    consts[:, 0] = inv_wsum
    consts[:, 1] = neg_thr
    return [
        {
            "x": np.ascontiguousarray(x[c * ROWS_PER_CORE : (c + 1) * ROWS_PER_CORE]),
            "w2": w2,
            "consts": consts,
        }
        for c in range(N_CORES)
    ]


def _gather(results):
    # y core tile is [ROWS_T, N_TILES]: y[p, t] = shard row t*ROWS_T + p
    # (the last tile only fills LAST_ROWS rows; the rest is padding).
    parts = [
        np.asarray(results[c]["y"]).T.reshape(-1)[:ROWS_PER_CORE]
        for c in range(N_CORES)
    ]
    return np.concatenate(parts).reshape(BATCH, 1).astype(np.float32)


def _run(x, wc, wint, thr, trace=False):
    from concourse import bass_utils

    nc = _get_program()
    in_maps = _make_in_maps(x, wc, wint, thr)
    res = bass_utils.run_bass_kernel_spmd(
        nc, in_maps, core_ids=list(range(N_CORES)), trace=trace
    )
    return _gather(res.results), res


def kernel(x, wc, wint, thr):
    out, _ = _run(x, wc, wint, thr, trace=False)
    return out
